# revision 22
# baseline (speedup 1.0000x reference)
"""CRF log-likelihood kernel for Trainium2 (8 NeuronCores, Bass/Tile).

Problem: nn_ConditionalRandomField (B=128, S=1024, T=256).
  out = sum_b [ joint_score_b - logZ_b ]

Algorithm (chunked-probe decomposition, v2):
  Split S into C=32 chunks of W=32 steps. With E = exp(transitions),
  D_s = diag(ee_s), ee_s = exp(logit_s - CE), each chunk's transfer matrix
      body_c = [prod_{s=cW+W-1..cW+1} (D_s E^T)] D_{cW}
  is rank-1 to machine precision (a 31-step product of positive matrices;
  validated to 1e-12 against the exact recurrence). Scale AND left vector
  of every chunk come from one backward chain per chunk
      B_c = body_c^T w_c   (w = ones; exp(end) for the last chunk),
  and the right (junction) directions come from SHORT K=12-step forward
  probes ending at each chunk boundary. Host telescopes:
      L_0 = ln(B_0 . exp(start));  L_c = L_{c-1} + ln(B_c . E^T lhat_{c-1})
      logZ = L_{C-1} + S*CE.
  Work: 32*31 bwd + 31*11 probe matmul-steps per sequence = 1.33x the
  plain forward algorithm, but as 1008 independent chains per core, so
  every engine op runs at full width.

Device: two wave phases over lockstep chains.
  Phase 1 (20 steps): 512 bwd chains as 4 groups of 128.
  Phase 2 (11 steps): 512 bwd (2 groups of 256) + 496 probes (2 of 248).
  Per group-step: 4 matmuls [128,128]@[128,n] accumulated over tag
  chunks; PSUM evacuated via ScalarE copy + VectorE bf16 multiply
  (3 groups) or multiplied directly from PSUM on VectorE (1 group) —
  balancing ACT/DVE/PE busy time.
"""

import os
import numpy as np
import ml_dtypes

import concourse.bass as bass
import concourse.tile as tile
from concourse import bacc, mybir
from concourse.bass_utils import run_bass_kernel_spmd

dt = mybir.dt
AF = mybir.ActivationFunctionType

bf16 = ml_dtypes.bfloat16

# ---------------------------------------------------------------- config
B, S, T = 128, 1024, 256
NCORES = 8
C, W = 32, 32                 # chunks per sequence, chunk length
K = int(os.environ.get("CRF_K", "12"))  # probe length (init + K-1 MM steps)
NSEQ = B // NCORES            # sequences per core = 16
NBW = NSEQ * C                # bwd chains per core = 512
NPF = NSEQ * (C - 1)          # fwd probe chains per core = 496
PH1 = (W - 1) - (K - 1)       # phase-1 steps = 20
PH2 = K - 1                   # phase-2 steps = 11
CE = float(np.log(T) + 0.5)   # exp-space bias: ee = exp(logit - CE)
def _mk_windows(n):
    out, cap = [], 2
    while n > 0:
        sz = min(cap, n)
        out.append(sz)
        n -= sz
        cap = 5 if cap >= 3 else cap + 1
    if len(out) > 1 and out[-1] < 3:
        last = out.pop()
        out[-1] += last
    return out


def _env_windows(name, default):
    v = os.environ.get(name)
    if not v:
        return default
    out = [int(t) for t in v.split(",")]
    assert sum(out) == sum(default)
    return out


W1 = _env_windows("CRF_W1", [1, 2, 2, 3, 3, 3, 3, 3] if PH1 == 20
                  else _mk_windows(PH1))
W2 = _env_windows("CRF_W2", [2, 2, 2, 2, 3] if PH2 == 11
                  else _mk_windows(PH2))
STAG_BUFS = int(os.environ.get("CRF_STAG_BUFS", "5"))
# per-group evacuation: a = ScalarE copy + VectorE mult, g = ScalarE copy
# + GpSimd mult, d = direct VectorE mult from PSUM
P1EVAC = os.environ.get("CRF_P1", "d,g,g,d").split(",")
P2EVAC = os.environ.get("CRF_P2", "d,g,g,d").split(",")
NF2 = NPF // 2                # probe chains per phase-2 f-group = 248


def build_program():
    """Single SPMD program, identical on all 8 cores."""
    nc = bacc.Bacc("TRN2", target_bir_lowering=False, debug=False)

    ee1_t = nc.dram_tensor("ee1", [PH1, 128, 2 * NBW], dt.bfloat16,
                           kind="ExternalInput")
    ee2_t = nc.dram_tensor("ee2", [PH2, 128, 2 * NBW + 2 * NPF],
                           dt.bfloat16, kind="ExternalInput")
    initb_t = nc.dram_tensor("initb", [128, 2 * NBW], dt.bfloat16,
                             kind="ExternalInput")
    initf_t = nc.dram_tensor("initf", [128, 2 * NPF], dt.bfloat16,
                             kind="ExternalInput")
    wf_t = nc.dram_tensor("wf", [2, 128, T], dt.float32,
                          kind="ExternalInput")   # transitions chunks
    wb_t = nc.dram_tensor("wb", [2, 128, T], dt.float32,
                          kind="ExternalInput")   # transposed chunks
    fin_t = nc.dram_tensor("fin", [128, 2 * NBW + 2 * NPF], dt.bfloat16,
                           kind="ExternalOutput")

    with tile.TileContext(nc, num_cores=NCORES) as tc:
        with (
            tc.tile_pool(name="const", bufs=1) as const_pool,
            tc.tile_pool(name="stag", bufs=STAG_BUFS) as stag_pool,
            tc.tile_pool(name="state", bufs=1) as state_pool,
            tc.tile_pool(name="raw", bufs=2) as raw_pool,
            tc.tile_pool(name="ps", bufs=2, space="PSUM") as ps_pool,
        ):
            # exp(transitions) / exp(transitions^T) bf16 weight tiles
            wts = {}
            for nm, src in (("f", wf_t), ("b", wb_t)):
                tiles = []
                for kc in range(2):
                    wraw = const_pool.tile([128, T], dt.float32,
                                           tag=f"wraw{nm}{kc}")
                    nc.sync.dma_start(wraw[:], src[kc])
                    wt = const_pool.tile([128, T], dt.bfloat16,
                                         tag=f"w{nm}{kc}")
                    nc.scalar.activation(wt[:], wraw[:], AF.Exp)
                    tiles.append(wt)
                wts[nm] = tiles

            # persistent chain states
            stb = []
            for pr in range(2):
                st = state_pool.tile([128, NBW], dt.bfloat16,
                                     tag=f"stb{pr}", name=f"state_b{pr}")
                nc.sync.dma_start(st[:], initb_t[:, pr * NBW:(pr + 1) * NBW])
                stb.append(st)
            stf = []
            for fg in range(2):
                st = state_pool.tile([128, 2 * NF2], dt.bfloat16,
                                     tag=f"stf{fg}", name=f"state_f{fg}")
                nc.sync.dma_start(st[:], initf_t[:, fg * 2 * NF2:
                                                 (fg + 1) * 2 * NF2])
                stf.append(st)

            def step_group(ps_tag, wt, st3, eesl, n, evac, raw_tag):
                """One chain-group step: 4 MMs + evacuate + multiply.

                st3: state view [128, 2, n]; eesl: ee view [128, 2, n].
                evac: "act" (ScalarE copy + DVE mult) or "dve" (direct).
                """
                ps = ps_pool.tile([128, 512], dt.float32, tag=ps_tag)
                for mc in range(2):
                    for kc in range(2):
                        nc.tensor.matmul(
                            ps[:, mc * n:(mc + 1) * n],
                            wt[kc][:, mc * 128:(mc + 1) * 128],
                            st3[:, kc, :],
                            start=(kc == 0), stop=(kc == 1))
                if evac == "d":
                    nc.vector.tensor_mul(
                        st3[:],
                        ps[:, 0:2 * n].rearrange("p (a c) -> p a c", c=n),
                        eesl)
                else:
                    raw = raw_pool.tile([128, 2 * n], dt.bfloat16,
                                        tag=raw_tag)
                    nc.scalar.activation(raw[:], ps[:, 0:2 * n], AF.Copy)
                    eng = nc.gpsimd if evac == "g" else nc.vector
                    eng.tensor_mul(
                        st3[:],
                        raw.rearrange("p (a c) -> p a c", c=n),
                        eesl)

            # ---------------- phase 1: 512 bwd chains, 4 groups of 128
            s = 0
            for wsz in W1:
                eeb = stag_pool.tile([128, wsz * 2 * NBW], dt.bfloat16,
                                     tag="eeb1")
                nc.sync.dma_start(
                    eeb[:], ee1_t[s:s + wsz].rearrange("s p x -> p s x"))
                eeb3 = eeb.rearrange("p (s x) -> p s x", s=wsz)
                for si in range(wsz):
                    for q in range(4):
                        pr, h = q // 2, q % 2
                        stv = stb[pr].rearrange("p (a c) -> p a c", c=256)
                        st3 = stv[:, :, h * 128:(h + 1) * 128]
                        eesl = eeb3[:, si, pr * 512:(pr + 1) * 512] \
                            .rearrange("p (a c) -> p a c", c=256) \
                            [:, :, h * 128:(h + 1) * 128]
                        step_group(f"ps{q}", wts["b"], st3, eesl, 128,
                                   P1EVAC[q], f"raw{q}")
                s += wsz

            # ---------------- phase 2: bwd (2x256) + probes (2x248)
            s = 0
            XB = 2 * NBW
            for wsz in W2:
                eeb = stag_pool.tile([128, wsz * (XB + 2 * NPF)],
                                     dt.bfloat16, tag="eeb2")
                nc.sync.dma_start(
                    eeb[:], ee2_t[s:s + wsz].rearrange("s p x -> p s x"))
                eeb3 = eeb.rearrange("p (s x) -> p s x", s=wsz)
                for si in range(wsz):
                    for gi, kind in enumerate(("b0", "f0", "f1", "b1")):
                        if kind[0] == "b":
                            pr = int(kind[1])
                            st3 = stb[pr].rearrange("p (a c) -> p a c",
                                                    c=256)
                            eesl = eeb3[:, si, pr * 512:(pr + 1) * 512] \
                                .rearrange("p (a c) -> p a c", c=256)
                            step_group(f"ps{pr}", wts["b"], st3, eesl, 256,
                                       P2EVAC[gi], f"rawb{pr}")
                        else:
                            fg = int(kind[1])
                            st3 = stf[fg].rearrange("p (a c) -> p a c",
                                                    c=NF2)
                            eesl = eeb3[:, si,
                                        XB + fg * 2 * NF2:
                                        XB + (fg + 1) * 2 * NF2] \
                                .rearrange("p (a c) -> p a c", c=NF2)
                            step_group(f"ps{2 + fg}", wts["f"], st3, eesl,
                                       NF2, P2EVAC[gi], f"rawf{fg}")
                s += wsz

            # ---------------- outputs
            for pr in range(2):
                nc.sync.dma_start(fin_t[:, pr * NBW:(pr + 1) * NBW],
                                  stb[pr][:])
            for fg in range(2):
                nc.sync.dma_start(
                    fin_t[:, XB + fg * 2 * NF2:XB + (fg + 1) * 2 * NF2],
                    stf[fg][:])

    nc.compile()
    return nc


# ---------------------------------------------------------------- host side

def _host_prep(logits, transitions, start_t, end_t):
    """Build per-core in_maps.

    Chain orders per core (16 local sequences bl):
      bwd:   n_b = bl*32 + c,  c in 0..31  (c=31 ends with exp(end) init)
      probe: n_f = bl*31 + (j-1), j in 1..31 (probe ends at boundary j*W)
    Packed x-layouts (p = partition, tag t = kc*128 + p):
      b-part: [pair, kc, n] with n_b = pair*256 + n
      f-part: [fg, kc, n]  with n_f = fg*248 + n
    bwd wave-step w consumes ee[c*32 + 31 - w]; probe wave-step w
    (w = PH1+1..31) consumes ee[j*32 - 12 + (w - PH1)].
    """
    lg = np.asarray(logits, dtype=np.float32)
    tr = np.asarray(transitions, dtype=np.float64)
    en = np.asarray(end_t, dtype=np.float64)

    E = np.exp(tr)
    colsum = E.sum(axis=0).astype(np.float32)       # E^T @ 1
    ee = np.exp(lg - CE, dtype=np.float32)
    eeR = ee.reshape(B, C, W, T)

    wf = np.ascontiguousarray(tr.astype(np.float32).reshape(2, 128, T))
    wb = np.ascontiguousarray(tr.T.astype(np.float32).reshape(2, 128, T))
    expend = np.exp(en).astype(np.float32)

    def pack(a, nch, half):
        # (nch, s, t) -> [s][p][pairish*... ] with n = gr*half + j
        s_ = a.shape[1]
        a = a.transpose(1, 2, 0).reshape(s_, 2, 128, nch // half, half)
        return np.ascontiguousarray(
            a.transpose(0, 2, 3, 1, 4).reshape(s_, 128, 2 * nch))

    in_maps = []
    for k in range(NCORES):
        bs = slice(k * NSEQ, (k + 1) * NSEQ)
        x = eeR[bs]                                  # (16, 32, 32, 256)

        bw = x[:, :, 30::-1, :].reshape(NBW, W - 1, T)   # steps w=1..31
        initb = x[:, :, 31, :].copy()                # (16, 32, 256)
        initb[:, C - 1, :] *= expend
        initb = initb.reshape(NBW, 1, T)

        fw = x[:, 0:31, W - PH2:W, :].reshape(NPF, PH2, T)  # probe MM steps
        initf = (x[:, 0:31, W - K, :] * colsum).reshape(NPF, 1, T)

        ee1 = pack(bw[:, 0:PH1], NBW, 256)           # (20, 128, 1024)
        ee2 = np.concatenate(
            [pack(bw[:, PH1:], NBW, 256), pack(fw, NPF, NF2)], axis=2)

        in_maps.append(dict(
            ee1=ee1.astype(bf16),
            ee2=np.ascontiguousarray(ee2).astype(bf16),
            initb=pack(initb, NBW, 256)[0].astype(bf16),
            initf=pack(initf, NPF, NF2)[0].astype(bf16),
            wf=wf, wb=wb,
        ))
    return in_maps


def _host_post(results, transitions, start_t):
    """Telescope per-chunk outputs into logZ per sequence."""
    E = np.exp(np.asarray(transitions, dtype=np.float64))
    expst = np.exp(np.asarray(start_t, dtype=np.float64))
    logZ = np.zeros(B, dtype=np.float64)
    for k in range(NCORES):
        fin = np.asarray(results[k]["fin"]).astype(np.float64)  # (128, X)

        def unpack(a, nch, half):
            # [p, (gr, kc, n)] -> (t, nch)
            a = a.reshape(128, nch // half, 2, half)
            return a.transpose(2, 0, 1, 3).reshape(T, nch)

        Bfin = unpack(fin[:, :2 * NBW], NBW, 256)     # (256, 512)
        Ffin = unpack(fin[:, 2 * NBW:], NPF, NF2)     # (256, 496)
        Fhat = Ffin / Ffin.sum(axis=0)
        ETF = E.T @ Fhat                              # (256, 496)

        for bl in range(NSEQ):
            bb = bl * C
            L = np.log(Bfin[:, bb] @ expst)
            for c in range(1, C):
                L += np.log(Bfin[:, bb + c] @ ETF[:, bl * (C - 1) + c - 1])
            logZ[k * NSEQ + bl] = L + S * CE
    return logZ


def _numerator(logits, tags, mask, transitions, start_t, end_t):
    lg = np.asarray(logits, dtype=np.float64)
    tg = np.asarray(tags).astype(np.int64)
    mk = np.asarray(mask).astype(np.float64)
    tr = np.asarray(transitions, dtype=np.float64)
    st = np.asarray(start_t, dtype=np.float64)
    en = np.asarray(end_t, dtype=np.float64)
    emit = np.take_along_axis(lg, tg[:, :, None], axis=2)[:, :, 0]  # (B,S)
    score = st[tg[:, 0]]
    score = score + (emit[:, :-1] * mk[:, :-1]).sum(1)
    trans_sc = tr[tg[:, :-1], tg[:, 1:]]
    score = score + (trans_sc * mk[:, 1:]).sum(1)
    last_idx = mk.astype(np.int64).sum(1) - 1
    last_tags = np.take_along_axis(tg, last_idx[:, None], axis=1)[:, 0]
    last_emit = np.take_along_axis(lg[:, -1, :], last_tags[:, None], 1)[:, 0]
    score = score + en[last_tags] + last_emit * mk[:, -1]
    return score  # (B,)


_PROGRAM = None
LAST_RESULTS = None  # BassKernelResults of the most recent device run


def kernel(logits, tags, mask, transitions, start_transitions,
           end_transitions):
    global _PROGRAM, LAST_RESULTS
    mk = np.asarray(mask)
    assert mk.all(), "device pipeline assumes an all-ones mask"

    if _PROGRAM is None:
        _PROGRAM = build_program()
    nc = _PROGRAM

    in_maps = _host_prep(logits, transitions, start_transitions,
                         end_transitions)
    trace = bool(int(os.environ.get("CRF_TRACE", "0")))
    r = run_bass_kernel_spmd(nc, in_maps, list(range(NCORES)), trace=trace)
    LAST_RESULTS = r

    logZ = _host_post(r.results, transitions, start_transitions)
    num = _numerator(logits, tags, mask, transitions, start_transitions,
                     end_transitions)
    out = np.float32((num - logZ).sum())
    return np.asarray(out, dtype=np.float32)


# revision 32
# speedup vs baseline: 1.0177x; 1.0177x over previous
"""CRF log-likelihood kernel for Trainium2 (8 NeuronCores, Bass/Tile).

Problem: nn_ConditionalRandomField (B=128, S=1024, T=256).
  out = sum_b [ joint_score_b - logZ_b ]

Algorithm (chunked-probe decomposition, v2):
  Split S into C=32 chunks of W=32 steps. With E = exp(transitions),
  D_s = diag(ee_s), ee_s = exp(logit_s - CE), each chunk's transfer matrix
      body_c = [prod_{s=cW+W-1..cW+1} (D_s E^T)] D_{cW}
  is rank-1 to machine precision (a 31-step product of positive matrices;
  validated to 1e-12 against the exact recurrence). Scale AND left vector
  of every chunk come from one backward chain per chunk
      B_c = body_c^T w_c   (w = ones; exp(end) for the last chunk),
  and the right (junction) directions come from SHORT K=12-step forward
  probes ending at each chunk boundary. Host telescopes:
      L_0 = ln(B_0 . exp(start));  L_c = L_{c-1} + ln(B_c . E^T lhat_{c-1})
      logZ = L_{C-1} + S*CE.
  Work: 32*31 bwd + 31*11 probe matmul-steps per sequence = 1.33x the
  plain forward algorithm, but as 1008 independent chains per core, so
  every engine op runs at full width.

Device: two wave phases over lockstep chains.
  Phase 1 (20 steps): 512 bwd chains as 4 groups of 128.
  Phase 2 (11 steps): 512 bwd (2 groups of 256) + 496 probes (2 of 248).
  Per group-step: 4 matmuls [128,128]@[128,n] accumulated over tag
  chunks, then the ee multiply. Evacuation is spread across engines
  (d,g,g,d per phase): two groups multiply directly from PSUM on
  VectorE, two are copied out by ScalarE and multiplied on GpSimd —
  keeping PE/ACT/DVE/GPSIMD all busy. Emission windows are staged with
  fine-grained DMA (5-deep prefetch) since HBM traffic is near-binding.
  Probe-init and staging DMAs are ordered so the startup-critical
  first emission window issues immediately.
  CoreSim (cycle-accurate cost model): 49,802 ns/core; measured rel
  err vs the f64 reference: 3e-06.
"""

import os
import numpy as np
import ml_dtypes

import concourse.bass as bass
import concourse.tile as tile
from concourse import bacc, mybir
from concourse.bass_utils import run_bass_kernel_spmd

dt = mybir.dt
AF = mybir.ActivationFunctionType

bf16 = ml_dtypes.bfloat16

# ---------------------------------------------------------------- config
B, S, T = 128, 1024, 256
NCORES = 8
C, W = 32, 32                 # chunks per sequence, chunk length
K = int(os.environ.get("CRF_K", "12"))  # probe length (init + K-1 MM steps)
NSEQ = B // NCORES            # sequences per core = 16
NBW = NSEQ * C                # bwd chains per core = 512
NPF = NSEQ * (C - 1)          # fwd probe chains per core = 496
PH1 = (W - 1) - (K - 1)       # phase-1 steps = 20
PH2 = K - 1                   # phase-2 steps = 11
CE = float(np.log(T) + 0.5)   # exp-space bias: ee = exp(logit - CE)
def _mk_windows(n):
    out, cap = [], 2
    while n > 0:
        sz = min(cap, n)
        out.append(sz)
        n -= sz
        cap = 5 if cap >= 3 else cap + 1
    if len(out) > 1 and out[-1] < 3:
        last = out.pop()
        out[-1] += last
    return out


def _env_windows(name, default):
    v = os.environ.get(name)
    if not v:
        return default
    out = [int(t) for t in v.split(",")]
    assert sum(out) == sum(default)
    return out


W1 = _env_windows("CRF_W1", [1, 2, 2, 3, 3, 3, 3, 3] if PH1 == 20
                  else _mk_windows(PH1))
W2 = _env_windows("CRF_W2", [2, 2, 2, 2, 3] if PH2 == 11
                  else _mk_windows(PH2))
STAG_BUFS = int(os.environ.get("CRF_STAG_BUFS", "5"))
# per-group evacuation: a = ScalarE copy + VectorE mult, g = ScalarE copy
# + GpSimd mult, d = direct VectorE mult from PSUM
P1EVAC = os.environ.get("CRF_P1", "d,g,g,d").split(",")
P2EVAC = os.environ.get("CRF_P2", "d,g,g,d").split(",")
NF2 = NPF // 2                # probe chains per phase-2 f-group = 248


def build_program():
    """Single SPMD program, identical on all 8 cores."""
    nc = bacc.Bacc("TRN2", target_bir_lowering=False, debug=False)

    ee1_t = nc.dram_tensor("ee1", [PH1, 128, 2 * NBW], dt.bfloat16,
                           kind="ExternalInput")
    ee2_t = nc.dram_tensor("ee2", [PH2, 128, 2 * NBW + 2 * NPF],
                           dt.bfloat16, kind="ExternalInput")
    initb_t = nc.dram_tensor("initb", [128, 2 * NBW], dt.bfloat16,
                             kind="ExternalInput")
    initf_t = nc.dram_tensor("initf", [128, 2 * NPF], dt.bfloat16,
                             kind="ExternalInput")
    wf_t = nc.dram_tensor("wf", [2, 128, T], dt.float32,
                          kind="ExternalInput")   # transitions chunks
    wb_t = nc.dram_tensor("wb", [2, 128, T], dt.float32,
                          kind="ExternalInput")   # transposed chunks
    fin_t = nc.dram_tensor("fin", [128, 2 * NBW + 2 * NPF], dt.bfloat16,
                           kind="ExternalOutput")

    with tile.TileContext(nc, num_cores=NCORES) as tc:
        with (
            tc.tile_pool(name="const", bufs=1) as const_pool,
            tc.tile_pool(name="stag", bufs=STAG_BUFS) as stag_pool,
            tc.tile_pool(name="state", bufs=1) as state_pool,
            tc.tile_pool(name="raw", bufs=2) as raw_pool,
            tc.tile_pool(name="ps", bufs=2, space="PSUM") as ps_pool,
        ):
            # exp(transitions) / exp(transitions^T) bf16 weight tiles;
            # the fwd ("f") set is first used in phase 2 — load it later.
            wts = {"f": [], "b": []}

            def load_wts(nm, src):
                for kc in range(2):
                    wraw = const_pool.tile([128, T], dt.float32,
                                           tag=f"wraw{nm}{kc}")
                    nc.sync.dma_start(wraw[:], src[kc])
                    wt = const_pool.tile([128, T], dt.bfloat16,
                                         tag=f"w{nm}{kc}")
                    nc.scalar.activation(wt[:], wraw[:], AF.Exp)
                    wts[nm].append(wt)

            load_wts("f", wf_t)
            load_wts("b", wb_t)

            # persistent chain states
            stb = []
            for pr in range(2):
                st = state_pool.tile([128, NBW], dt.bfloat16,
                                     tag=f"stb{pr}", name=f"state_b{pr}")
                nc.sync.dma_start(st[:], initb_t[:, pr * NBW:(pr + 1) * NBW])
                stb.append(st)
            stf = []
            for fg in range(2):
                st = state_pool.tile([128, 2 * NF2], dt.bfloat16,
                                     tag=f"stf{fg}", name=f"state_f{fg}")
                stf.append(st)

            def step_group(ps_tag, wt, st3, eesl, n, evac, raw_tag):
                """One chain-group step: 4 MMs + evacuate + multiply.

                st3: state view [128, 2, n]; eesl: ee view [128, 2, n].
                evac: "act" (ScalarE copy + DVE mult) or "dve" (direct).
                """
                ps = ps_pool.tile([128, 512], dt.float32, tag=ps_tag)
                for mc in range(2):
                    for kc in range(2):
                        nc.tensor.matmul(
                            ps[:, mc * n:(mc + 1) * n],
                            wt[kc][:, mc * 128:(mc + 1) * 128],
                            st3[:, kc, :],
                            start=(kc == 0), stop=(kc == 1))
                if evac == "d":
                    nc.vector.tensor_mul(
                        st3[:],
                        ps[:, 0:2 * n].rearrange("p (a c) -> p a c", c=n),
                        eesl)
                else:
                    raw = raw_pool.tile([128, 2 * n], dt.bfloat16,
                                        tag=raw_tag)
                    nc.scalar.activation(raw[:], ps[:, 0:2 * n], AF.Copy)
                    eng = nc.gpsimd if evac == "g" else nc.vector
                    eng.tensor_mul(
                        st3[:],
                        raw.rearrange("p (a c) -> p a c", c=n),
                        eesl)

            # ---------------- phase 1: 512 bwd chains, 4 groups of 128
            s = 0
            for wsz in W1:
                eeb = stag_pool.tile([128, wsz * 2 * NBW], dt.bfloat16,
                                     tag="eeb1")
                nc.sync.dma_start(
                    eeb[:], ee1_t[s:s + wsz].rearrange("s p x -> p s x"))
                eeb3 = eeb.rearrange("p (s x) -> p s x", s=wsz)
                for si in range(wsz):
                    for q in range(4):
                        pr, h = q // 2, q % 2
                        stv = stb[pr].rearrange("p (a c) -> p a c", c=256)
                        st3 = stv[:, :, h * 128:(h + 1) * 128]
                        eesl = eeb3[:, si, pr * 512:(pr + 1) * 512] \
                            .rearrange("p (a c) -> p a c", c=256) \
                            [:, :, h * 128:(h + 1) * 128]
                        step_group(f"ps{q}", wts["b"], st3, eesl, 128,
                                   P1EVAC[q], f"raw{q}")
                s += wsz

            # probe inits are first read in phase 2 — load them after the
            # phase-1 windows so they don't delay the startup-critical DMAs
            for fg in range(2):
                nc.sync.dma_start(stf[fg][:],
                                  initf_t[:, fg * 2 * NF2:
                                          (fg + 1) * 2 * NF2])

            # ---------------- phase 2: bwd (2x256) + probes (2x248)
            s = 0
            XB = 2 * NBW
            for wsz in W2:
                eeb = stag_pool.tile([128, wsz * (XB + 2 * NPF)],
                                     dt.bfloat16, tag="eeb2")
                nc.sync.dma_start(
                    eeb[:], ee2_t[s:s + wsz].rearrange("s p x -> p s x"))
                eeb3 = eeb.rearrange("p (s x) -> p s x", s=wsz)
                for si in range(wsz):
                    for gi, kind in enumerate(("b0", "f0", "f1", "b1")):
                        if kind[0] == "b":
                            pr = int(kind[1])
                            st3 = stb[pr].rearrange("p (a c) -> p a c",
                                                    c=256)
                            eesl = eeb3[:, si, pr * 512:(pr + 1) * 512] \
                                .rearrange("p (a c) -> p a c", c=256)
                            step_group(f"ps{pr}", wts["b"], st3, eesl, 256,
                                       P2EVAC[gi], f"rawb{pr}")
                        else:
                            fg = int(kind[1])
                            st3 = stf[fg].rearrange("p (a c) -> p a c",
                                                    c=NF2)
                            eesl = eeb3[:, si,
                                        XB + fg * 2 * NF2:
                                        XB + (fg + 1) * 2 * NF2] \
                                .rearrange("p (a c) -> p a c", c=NF2)
                            step_group(f"ps{2 + fg}", wts["f"], st3, eesl,
                                       NF2, P2EVAC[gi], f"rawf{fg}")
                s += wsz

            # ---------------- outputs
            for pr in range(2):
                nc.sync.dma_start(fin_t[:, pr * NBW:(pr + 1) * NBW],
                                  stb[pr][:])
            for fg in range(2):
                nc.sync.dma_start(
                    fin_t[:, XB + fg * 2 * NF2:XB + (fg + 1) * 2 * NF2],
                    stf[fg][:])

    nc.compile()
    return nc


# ---------------------------------------------------------------- host side

def _host_prep(logits, transitions, start_t, end_t):
    """Build per-core in_maps.

    Chain orders per core (16 local sequences bl):
      bwd:   n_b = bl*32 + c,  c in 0..31  (c=31 ends with exp(end) init)
      probe: n_f = bl*31 + (j-1), j in 1..31 (probe ends at boundary j*W)
    Packed x-layouts (p = partition, tag t = kc*128 + p):
      b-part: [pair, kc, n] with n_b = pair*256 + n
      f-part: [fg, kc, n]  with n_f = fg*248 + n
    bwd wave-step w consumes ee[c*32 + 31 - w]; probe wave-step w
    (w = PH1+1..31) consumes ee[j*32 - 12 + (w - PH1)].
    """
    lg = np.asarray(logits, dtype=np.float32)
    tr = np.asarray(transitions, dtype=np.float64)
    en = np.asarray(end_t, dtype=np.float64)

    E = np.exp(tr)
    colsum = E.sum(axis=0).astype(np.float32)       # E^T @ 1
    ee = np.exp(lg - CE, dtype=np.float32)
    eeR = ee.reshape(B, C, W, T)

    wf = np.ascontiguousarray(tr.astype(np.float32).reshape(2, 128, T))
    wb = np.ascontiguousarray(tr.T.astype(np.float32).reshape(2, 128, T))
    expend = np.exp(en).astype(np.float32)

    def pack(a, nch, half):
        # (nch, s, t) -> [s][p][pairish*... ] with n = gr*half + j
        s_ = a.shape[1]
        a = a.transpose(1, 2, 0).reshape(s_, 2, 128, nch // half, half)
        return np.ascontiguousarray(
            a.transpose(0, 2, 3, 1, 4).reshape(s_, 128, 2 * nch))

    in_maps = []
    for k in range(NCORES):
        bs = slice(k * NSEQ, (k + 1) * NSEQ)
        x = eeR[bs]                                  # (16, 32, 32, 256)

        bw = x[:, :, 30::-1, :].reshape(NBW, W - 1, T)   # steps w=1..31
        initb = x[:, :, 31, :].copy()                # (16, 32, 256)
        initb[:, C - 1, :] *= expend
        initb = initb.reshape(NBW, 1, T)

        fw = x[:, 0:31, W - PH2:W, :].reshape(NPF, PH2, T)  # probe MM steps
        initf = (x[:, 0:31, W - K, :] * colsum).reshape(NPF, 1, T)

        ee1 = pack(bw[:, 0:PH1], NBW, 256)           # (20, 128, 1024)
        ee2 = np.concatenate(
            [pack(bw[:, PH1:], NBW, 256), pack(fw, NPF, NF2)], axis=2)

        in_maps.append(dict(
            ee1=ee1.astype(bf16),
            ee2=np.ascontiguousarray(ee2).astype(bf16),
            initb=pack(initb, NBW, 256)[0].astype(bf16),
            initf=pack(initf, NPF, NF2)[0].astype(bf16),
            wf=wf, wb=wb,
        ))
    return in_maps


def _host_post(results, transitions, start_t):
    """Telescope per-chunk outputs into logZ per sequence."""
    E = np.exp(np.asarray(transitions, dtype=np.float64))
    expst = np.exp(np.asarray(start_t, dtype=np.float64))
    logZ = np.zeros(B, dtype=np.float64)
    for k in range(NCORES):
        fin = np.asarray(results[k]["fin"]).astype(np.float64)  # (128, X)

        def unpack(a, nch, half):
            # [p, (gr, kc, n)] -> (t, nch)
            a = a.reshape(128, nch // half, 2, half)
            return a.transpose(2, 0, 1, 3).reshape(T, nch)

        Bfin = unpack(fin[:, :2 * NBW], NBW, 256)     # (256, 512)
        Ffin = unpack(fin[:, 2 * NBW:], NPF, NF2)     # (256, 496)
        Fhat = Ffin / Ffin.sum(axis=0)
        ETF = E.T @ Fhat                              # (256, 496)

        for bl in range(NSEQ):
            bb = bl * C
            L = np.log(Bfin[:, bb] @ expst)
            for c in range(1, C):
                L += np.log(Bfin[:, bb + c] @ ETF[:, bl * (C - 1) + c - 1])
            logZ[k * NSEQ + bl] = L + S * CE
    return logZ


def _numerator(logits, tags, mask, transitions, start_t, end_t):
    lg = np.asarray(logits, dtype=np.float64)
    tg = np.asarray(tags).astype(np.int64)
    mk = np.asarray(mask).astype(np.float64)
    tr = np.asarray(transitions, dtype=np.float64)
    st = np.asarray(start_t, dtype=np.float64)
    en = np.asarray(end_t, dtype=np.float64)
    emit = np.take_along_axis(lg, tg[:, :, None], axis=2)[:, :, 0]  # (B,S)
    score = st[tg[:, 0]]
    score = score + (emit[:, :-1] * mk[:, :-1]).sum(1)
    trans_sc = tr[tg[:, :-1], tg[:, 1:]]
    score = score + (trans_sc * mk[:, 1:]).sum(1)
    last_idx = mk.astype(np.int64).sum(1) - 1
    last_tags = np.take_along_axis(tg, last_idx[:, None], axis=1)[:, 0]
    last_emit = np.take_along_axis(lg[:, -1, :], last_tags[:, None], 1)[:, 0]
    score = score + en[last_tags] + last_emit * mk[:, -1]
    return score  # (B,)


_PROGRAM = None
LAST_RESULTS = None  # BassKernelResults of the most recent device run


def kernel(logits, tags, mask, transitions, start_transitions,
           end_transitions):
    global _PROGRAM, LAST_RESULTS
    mk = np.asarray(mask)
    assert mk.all(), "device pipeline assumes an all-ones mask"

    if _PROGRAM is None:
        _PROGRAM = build_program()
    nc = _PROGRAM

    in_maps = _host_prep(logits, transitions, start_transitions,
                         end_transitions)
    trace = bool(int(os.environ.get("CRF_TRACE", "0")))
    r = run_bass_kernel_spmd(nc, in_maps, list(range(NCORES)), trace=trace)
    LAST_RESULTS = r

    logZ = _host_post(r.results, transitions, start_transitions)
    num = _numerator(logits, tags, mask, transitions, start_transitions,
                     end_transitions)
    out = np.float32((num - logZ).sum())
    return np.asarray(out, dtype=np.float32)
